# revision 1
# baseline (speedup 1.0000x reference)
"""Trainium2 Bass kernel for nn_NodeAttentionPerMetaPath (GAT-style node attention).

Reference computation (N=8192, F_IN=256, d=64):
    h      = x @ trans                      # [N, d]
    e1     = h @ attn[:d];  e2 = h @ attn[d:]
    scores = leaky_relu(e1 + e2.T, 0.2)     # [N, N]
    masked = where(mask==0, -1e15, scores)
    out    = softmax(masked, axis=1) @ h    # [N, d]

Sharding: rows of mask/x across 8 cores (1024 rows each); h/e2 all-gathered.

Key algebraic restructuring (avoids any ACT pass over the [N,N] matrix):
    exp(leaky(v)) = max(exp(v), exp(a*v))          (exp monotone, a<1)
    with v = e1[r]+e2[j]:
      P = m * B2[j]*A2[r] * max(C[r]*D[j], 1)
    where C=exp((1-a)e1), D=exp((1-a)e2), A2=exp(a*e1), B2=exp(a*e2).
    The A2[r] factor cancels in softmax; B2[j] folds into h's rows:
      out = (P' @ h_scaled) / (P' @ B2),  P' = m * max(C[r]*D[j], 1)
    So the [N,N] work is ONE fused tensor_scalar (outer product + max with 1,
    4x bf16 DVE mode) and ONE masked multiply (split DVE/GPSIMD), then PE
    transposes into [j, r] layout and a 64-deep accumulated matmul whose
    extra column (=B2) yields the softmax denominator for free.
"""

import os
from contextlib import ExitStack

import numpy as np

import concourse.bass as bass
import concourse.bacc as bacc
import concourse.mybir as mybir
import concourse.tile as tile
from concourse.bass_utils import run_bass_kernel_spmd
from concourse.masks import make_identity

f32 = mybir.dt.float32
bf16 = mybir.dt.bfloat16
i32 = mybir.dt.int32
i16 = mybir.dt.int16
# score-pipeline dtype: fp16 has 10 mantissa bits (4x finer than bf16) and the
# whole pipeline stays in [2^-8, 2^12] — well inside fp16 range. Same speed.
SDT = mybir.dt.float16

Exp = mybir.ActivationFunctionType.Exp

N_CORES = 8
N = 8192
F_IN = 256
D = 64  # F_OUT
ALPHA = 0.2

R = N // N_CORES  # rows per core
RB = 128  # row-block
N_RB = R // RB  # row-blocks per core
GROUP = 2  # row-blocks per matmul group (moving N = GROUP*128)
N_GROUPS = N_RB // GROUP
JC = N // 128  # j-chunks

# TT mask-multiply split: columns [0:TT_SPLIT] on DVE, rest on GPSIMD
# (GPSIMD measured ~1.93 ns/col vs DVE ~1.09 at 1x; balance accordingly)
TT_SPLIT = 4608


def build_kernel(ctx: ExitStack, tc: tile.TileContext, x_rows, mask_c, trans, a12, outT):
    nc = tc.nc

    singles = ctx.enter_context(tc.tile_pool(name="singles", bufs=1))

    maskp = ctx.enter_context(tc.tile_pool(name="maskp", bufs=2))
    work = ctx.enter_context(tc.tile_pool(name="work", bufs=2))
    ptp = ctx.enter_context(tc.tile_pool(name="ptp", bufs=1))
    ps_t = ctx.enter_context(tc.tile_pool(name="ps_t", bufs=2, space="PSUM"))
    ps_o = ctx.enter_context(tc.tile_pool(name="ps_o", bufs=2, space="PSUM"))
    ps_r = ctx.enter_context(tc.tile_pool(name="ps_r", bufs=1, space="PSUM"))
    outp = ctx.enter_context(tc.tile_pool(name="outp", bufs=2))

    # mask stream issued FIRST (trace order drives tile's scheduling epoch):
    # HWDGE lanes belong to the mask from t=0; blocks beyond the buffer depth
    # pace themselves on tile-slot release by the consuming TTs
    mask_tiles = []
    for rb in range(N_RB):
        rows = slice(rb * 128, (rb + 1) * 128)
        m0 = maskp.tile([128, TT_SPLIT], i32, tag="m0", bufs=2)
        m1 = maskp.tile([128, N - TT_SPLIT], i32, tag="m1", bufs=2)
        nc.sync.dma_start(out=m0, in_=mask_c[rows, 0:TT_SPLIT])
        nc.sync.dma_start(out=m1, in_=mask_c[rows, TT_SPLIT:])
        mask_tiles.append((m0, m1))

    ident_b = singles.tile([128, 128], SDT)
    make_identity(nc, ident_b)
    ones_1x1 = singles.tile([1, 1], f32)
    nc.vector.memset(ones_1x1, 1.0)
    ones_row_f = singles.tile([1, D], f32)
    nc.vector.memset(ones_row_f, 1.0)

    # persistent steady-state tensors
    haug = singles.tile([128, JC, D + 1], SDT)  # [j%128, j//128, d | B2]
    d_rep = singles.tile([128, N], SDT)  # D[j] replicated over partitions
    c_own = singles.tile([128, N_RB], f32)  # C[r] for own rows


    # ---------------- phase 1: h/e on own rows, pre-scaled gather of haug/D
    with (
        tc.tile_pool(name="ph1", bufs=2) as ph1,
        tc.tile_pool(name="ph1s", bufs=1) as ph1s,
        tc.tile_pool(name="ph1ps", bufs=1, space="PSUM") as ph1ps,
        tc.tile_pool(name="dram", bufs=1, space="DRAM") as dram,
    ):
        shared = "Shared" if N_CORES > 4 else "Local"
        ident_f = ph1s.tile([128, 128], f32)
        make_identity(nc, ident_f)
        trans_sb = ph1s.tile([128, 2, D], f32)
        nc.gpsimd.dma_start(out=trans_sb, in_=trans.rearrange("(c p) d -> p c d", p=128))
        a12_sb = ph1s.tile([D, 2], f32)
        nc.gpsimd.dma_start(out=a12_sb, in_=a12[:, :])
        # tiny warm-up collective issued first: absorbs the one-time comm
        # init latency while phase-1 compute runs
        d_bnc_in = dram.tile([R, 1], SDT)
        d_bnc_out = dram.tile([N, 1], SDT, addr_space=shared)
        warm_in = dram.tile([1, 2], f32)
        warm_out = dram.tile([N_CORES, 2], f32, addr_space=shared)
        nc.gpsimd.dma_start(out=warm_in, in_=a12[0:1, :])
        groups = [list(range(N_CORES))]
        # warm-up barrier: absorbs comm-ring init + cross-core launch skew
        nc.gpsimd.collective_compute(
            "AllGather",
            mybir.AluOpType.bypass,
            replica_groups=groups,
            ins=[warm_in.opt()],
            outs=[warm_out.opt()],
        )

        # own x rows in one DMA, then PE transposes
        x_all = ph1s.tile([128, N_RB, F_IN], f32)
        nc.gpsimd.dma_start(
            out=x_all, in_=x_rows.rearrange("(c p) f -> p c f", p=128)
        )
        xT = ph1s.tile([128, 2, R], f32)
        for rc in range(N_RB):
            for fc in range(2):
                pt = ph1ps.tile([128, 128], f32, tag="ps_a", bufs=2)
                nc.tensor.transpose(
                    pt, x_all[:, rc, fc * 128 : (fc + 1) * 128], ident_f
                )
                nc.vector.tensor_copy(xT[:, fc, rc * 128 : (rc + 1) * 128], pt)

        # hT [d, r] = trans.T @ x_own.T
        hT = ph1s.tile([D, R], f32)
        for nb in range(R // 512):
            hps = ph1ps.tile([D, 512], f32, tag="ps_b")
            for fc in range(2):
                nc.tensor.matmul(
                    hps,
                    trans_sb[:, fc, :],
                    xT[:, fc, nb * 512 : (nb + 1) * 512],
                    start=(fc == 0),
                    stop=(fc == 1),
                )
            nc.vector.tensor_copy(hT[:, nb * 512 : (nb + 1) * 512], hps)

        # e1/e2 per-partition columns, directly: [128 r, 2] = hT_chunk.T @ a12
        e12_col = ph1s.tile([128, 2, N_RB], f32)
        for rc in range(N_RB):
            e_ps = ph1ps.tile([128, 2], f32, tag="ps_a", bufs=2)
            nc.tensor.matmul(
                e_ps,
                hT[:, rc * 128 : (rc + 1) * 128],
                a12_sb,
                start=True,
                stop=True,
            )
            nc.vector.tensor_copy(e12_col[:, :, rc], e_ps)

        nc.scalar.activation(c_own, e12_col[:, 0, :], Exp, scale=1.0 - ALPHA)
        b2_own = ph1s.tile([128, N_RB], f32)
        nc.scalar.activation(b2_own, e12_col[:, 1, :], Exp, scale=ALPHA)
        # D_own in column layout (fp16) for the gather: D = exp((1-a) e2)
        d_own_col = ph1s.tile([128, N_RB], SDT)
        nc.scalar.activation(d_own_col, e12_col[:, 1, :], Exp, scale=1.0 - ALPHA)

        # own haug rows: [j%128, rc, 0:64] = B2*h (from hT transposes), col 64 = B2
        haug_own = ph1s.tile([128, N_RB, D + 1], SDT)
        for rc in range(N_RB):
            hp = ph1ps.tile([128, D], f32, tag="ps_a", bufs=2)
            nc.tensor.transpose(
                hp, hT[:, rc * 128 : (rc + 1) * 128], ident_f[0:D, 0:D]
            )
            nc.vector.tensor_scalar(
                haug_own[:, rc, 0:D],
                hp,
                b2_own[:, rc : rc + 1],
                None,
                mybir.AluOpType.mult,
            )
        nc.vector.tensor_copy(haug_own[:, :, D], b2_own)

        # allgather pre-scaled haug rows + D row (both fp16)
        haug_bnc_in = dram.tile([R, D + 1], SDT)
        haug_bnc_out = dram.tile([N, D + 1], SDT, addr_space=shared)
        nc.gpsimd.dma_start(
            out=haug_bnc_in.rearrange("(c p) d -> p c d", p=128), in_=haug_own
        )
        nc.gpsimd.dma_start(
            out=d_bnc_in.rearrange("(c p) one -> p (c one)", p=128), in_=d_own_col
        )
        # d first: it gates the very first phase-2 op (v = C*D)
        nc.gpsimd.collective_compute(
            "AllGather",
            mybir.AluOpType.bypass,
            replica_groups=groups,
            ins=[d_bnc_in.opt()],
            outs=[d_bnc_out.opt()],
        )
        nc.gpsimd.collective_compute(
            "AllGather",
            mybir.AluOpType.bypass,
            replica_groups=groups,
            ins=[haug_bnc_in.opt()],
            outs=[haug_bnc_out.opt()],
        )

        # d_rep via partition-broadcast DMA straight from the gathered row
        d_flat = d_bnc_out.rearrange("n one -> (n one)")
        d_bcast = bass.AP(
            tensor=d_flat.tensor,
            offset=d_flat.offset,
            ap=[[0, 128], [1, N]],
        )
        nc.gpsimd.dma_start(out=d_rep, in_=d_bcast)
        # gathered haug -> sbuf in matmul-ready layout (one DMA)
        nc.gpsimd.dma_start(
            out=haug, in_=haug_bnc_out.rearrange("(j p) d -> p j d", p=128)
        )

    # ---------------- phase 2: streaming attention over row-blocks
    for g in range(N_GROUPS):
        pT = ptp.tile([128, JC, GROUP * 128], SDT, tag="pT")
        for rbi in range(GROUP):
            rb = g * GROUP + rbi
            m0, m1 = mask_tiles[rb]
            # low int16 halves of the int32 mask words: exact 0/1 values
            m0h = m0.bitcast(i16).rearrange("p (n two) -> p n two", two=2)[:, :, 0]
            m1h = m1.bitcast(i16).rearrange("p (n two) -> p n two", two=2)[:, :, 0]

            # v = max(C[r]*D[j], 1)   (single fused tensor_scalar, 4x fp16)
            v_t = work.tile([128, N], SDT, tag="v_t")
            nc.vector.tensor_scalar(
                v_t,
                d_rep,
                c_own[:, rb : rb + 1],
                1.0,
                mybir.AluOpType.mult,
                mybir.AluOpType.max,
            )
            # P' = mask * v, in place (split DVE / GPSIMD)
            p_t = v_t
            nc.vector.tensor_tensor(
                p_t[:, 0:TT_SPLIT], v_t[:, 0:TT_SPLIT], m0h, mybir.AluOpType.mult
            )
            nc.gpsimd.tensor_tensor(
                p_t[:, TT_SPLIT:], v_t[:, TT_SPLIT:], m1h, mybir.AluOpType.mult
            )

            # PE transpose P' into [j, r] layout, 4 chunks per PSUM tile
            for c4 in range(JC // 4):
                tp = ps_t.tile([128, 4, 128], SDT, tag="tp")
                for k in range(4):
                    ci = c4 * 4 + k
                    nc.tensor.transpose(
                        tp[:, k, :], p_t[:, ci * 128 : (ci + 1) * 128], ident_b
                    )
                dst = pT[:, c4 * 4 : (c4 + 1) * 4, rbi * 128 : (rbi + 1) * 128]
                if c4 % 2 == 0:
                    nc.vector.tensor_copy(dst, tp)
                else:
                    nc.scalar.copy(dst, tp)

        # accumulated matmul: out_aug.T[d|denom, r] = sum_j haug[j,:].T P'[j,r]
        po = ps_o.tile([D + 1, GROUP * 128], f32, tag="po")
        for ci in range(JC):
            nc.tensor.matmul(
                po, haug[:, ci, :], pT[:, ci, :], start=(ci == 0), stop=(ci == JC - 1)
            )

        # normalize: out = numer * (1/denom), denom broadcast via K=1 outer
        recip = outp.tile([1, GROUP * 128], f32, tag="recip")
        nc.vector.reciprocal(recip, po[D : D + 1, :])
        rr = ps_r.tile([D, GROUP * 128], f32, tag="rr")
        nc.tensor.matmul(rr, ones_row_f, recip, start=True, stop=True)
        rr_sb = outp.tile([D, GROUP * 128], f32, tag="rr_sb")
        nc.vector.tensor_copy(rr_sb, rr)
        o_t = outp.tile([D, GROUP * 128], f32, tag="o_t")
        nc.vector.tensor_tensor(o_t, po[0:D, :], rr_sb, mybir.AluOpType.mult)
        nc.gpsimd.dma_start(
            out=outT[:, g * GROUP * 128 : (g + 1) * GROUP * 128], in_=o_t
        )


def build_nc():
    nc = bacc.Bacc("TRN2", num_devices=N_CORES)
    x_rows = nc.dram_tensor("x_rows", [R, F_IN], f32, kind="ExternalInput")
    mask_c = nc.dram_tensor("mask_c", [R, N], i32, kind="ExternalInput")
    trans = nc.dram_tensor("trans", [F_IN, D], f32, kind="ExternalInput")
    a12 = nc.dram_tensor("a12", [D, 2], f32, kind="ExternalInput")
    outT = nc.dram_tensor("outT", [D, R], f32, kind="ExternalOutput")
    with ExitStack() as ctx:
        tc = ctx.enter_context(tile.TileContext(nc))
        build_kernel(ctx, tc, x_rows[:, :], mask_c[:, :], trans[:, :], a12[:, :], outT[:, :])
    nc.compile()
    return nc


LAST_RESULTS = None


def kernel(x, mask, trans, attn, _trace=False):
    x = np.ascontiguousarray(np.asarray(x), dtype=np.float32)
    mask = np.ascontiguousarray(np.asarray(mask), dtype=np.int32)
    trans = np.ascontiguousarray(np.asarray(trans), dtype=np.float32)
    attn = np.ascontiguousarray(np.asarray(attn), dtype=np.float32)
    a12 = np.ascontiguousarray(np.concatenate([attn[:D], attn[D:]], axis=1))

    nc = build_nc()
    in_maps = [
        {
            "x_rows": x[c * R : (c + 1) * R],
            "mask_c": mask[c * R : (c + 1) * R],
            "trans": trans,
            "a12": a12,
        }
        for c in range(N_CORES)
    ]
    res = run_bass_kernel_spmd(
        nc, in_maps, list(range(N_CORES)), trace=_trace
    )
    global LAST_RESULTS
    LAST_RESULTS = res
    out = np.concatenate(
        [res.results[c]["outT"].T for c in range(N_CORES)], axis=0
    )
    return np.ascontiguousarray(out, dtype=np.float32)


if __name__ == "__main__":
    nc = build_nc()
    print("built OK")



# revision 4
# speedup vs baseline: 2.0519x; 2.0519x over previous
"""Trainium2 Bass kernel for nn_NodeAttentionPerMetaPath (GAT-style node attention).

Reference computation (N=8192, F_IN=256, d=64):
    h      = x @ trans                      # [N, d]
    e1     = h @ attn[:d];  e2 = h @ attn[d:]
    scores = leaky_relu(e1 + e2.T, 0.2)     # [N, N]
    masked = where(mask==0, -1e15, scores)
    out    = softmax(masked, axis=1) @ h    # [N, d]

Design (v2 — collective-free, transpose-free):
  * exp(leaky(v)) = max(exp(v), exp(a*v)); with v = e1[r]+e2[j] and the
    per-row softmax invariances (drop exp(a*e1[r]), divide by C[r]):
        w''[j,r] = m[j,r] * B2[j] * max(D[j], invC[r])
    where D = exp(0.8*e2), invC = exp(-0.8*e1), B2 = exp(0.2*e2).
        out[r,:] = (P @ (B2*h)) / (P @ B2),  P[j,r] = m*max(D[j], invC[r])
  * Every core redundantly computes h/e2 for ALL N rows from a host-side
    x.T (fp16, 4MB) — removes both all-gathers and all cross-core skew.
  * The mask arrives host-transposed (and per-core row-permuted so "own"
    rows are always block 0), so all [N,N]-scale work happens directly in
    [j, r] layout: per 128-j chunk ONE tensor_scalar (4x fp16 DVE) + ONE
    mask tensor_tensor (2x DVE, some chunks on GpSimd) + two 512-moving
    accumulating matmuls. No PE transposes at all.
  * The [h | e2] trick: e2 = x @ (trans @ a2), so a single matmul with
    rhs = [trans | trans@a2] (65 cols) yields h and e2 together.
  * Output accumulates over all 64 j-chunks into 2 PSUM banks; the B2
    column of the augmented lhs yields the softmax denominator for free.
"""

from contextlib import ExitStack

import numpy as np

import concourse.bass as bass
import concourse.bacc as bacc
import concourse.mybir as mybir
import concourse.tile as tile
from concourse.bass_utils import run_bass_kernel_spmd

f32 = mybir.dt.float32
f16 = mybir.dt.float16
i32 = mybir.dt.int32
i8 = mybir.dt.int8

SDT = f16  # score-pipeline dtype
MDT = f16  # mask dtype on the wire (f16 -> 2x DVE TT; i8 -> half DMA, 1x TT)
MDT_NP = np.float16

Exp = mybir.ActivationFunctionType.Exp
MULT = mybir.AluOpType.mult
MAX = mybir.AluOpType.max

N_CORES = 8
N = 8192
F_IN = 256
D = 64
ALPHA = 0.2
R = N // N_CORES  # rows per core (1024)
NCH = N // 128  # j-chunks (64)
LAG = 8  # j-chunks of lag between phase-1 production and phase-2 consumption

# phase-2 mask-multiply split: every GP_EVERY-th chunk's TT runs on GpSimd
GP_EVERY = 3


def build_kernel(ctx: ExitStack, tc: tile.TileContext, xt, transp, transt, a12, maskt, outT):
    nc = tc.nc

    singles = ctx.enter_context(tc.tile_pool(name="singles", bufs=1))
    maskp = ctx.enter_context(tc.tile_pool(name="maskp", bufs=6))
    vp = ctx.enter_context(tc.tile_pool(name="vp", bufs=3))

    # ---------------- DMA issue (order drives queue order) ----------------
    # x.T pieces on the scalar-engine HWDGE queue: 8 x [128, 2, 1024] f16
    xt_tiles = []
    for p in range(8):
        t = singles.tile([128, 2, 1024], f16, tag=f"xt{p}")
        nc.scalar.dma_start(
            out=t,
            in_=xt[:, p * 1024 : (p + 1) * 1024].rearrange("(c p) n -> p c n", p=128),
        )
        xt_tiles.append(t)

    trans_sb = singles.tile([128, 2, D], f16, tag="trans")
    nc.gpsimd.dma_start(out=trans_sb, in_=transp.rearrange("(c p) d -> p c d", p=128))
    transT_sb = singles.tile([D, F_IN], f16, tag="transT")
    nc.gpsimd.dma_start(out=transT_sb, in_=transt[:, :])
    a12_sb = singles.tile([D, 2], f16, tag="a12")
    nc.gpsimd.dma_start(out=a12_sb, in_=a12[:, :])

    # mask chunk stream on the sync-engine queue (paced by tile-slot release)
    mask_tiles = []
    for k in range(NCH):
        m = maskp.tile([128, R], MDT, tag="m", bufs=6)
        nc.sync.dma_start(out=m, in_=maskt[k * 128 : (k + 1) * 128, :])
        mask_tiles.append(m)

    # ---------------- persistent sbuf ----------------
    haug = singles.tile([128, NCH, D + 1], SDT, tag="haug")  # [j%128, jc, B2*h | B2]
    d_all = singles.tile([128, NCH], f32, tag="d_all")
    b2_all = singles.tile([128, NCH], f32, tag="b2")
    invc_rep = singles.tile([128, R], SDT, tag="invc")
    invc_row = singles.tile([1, R], SDT, tag="invcr")
    ones128 = singles.tile([1, 128], f16, tag="ones128")
    nc.vector.memset(ones128, 1.0)
    ones64 = singles.tile([1, D], f32, tag="ones64")
    nc.vector.memset(ones64, 1.0)
    rhs2 = singles.tile([128, 2, D + 1], f16, tag="rhs2")  # [trans | trans@a2]
    ta12 = singles.tile([128, 2, 2], f16, tag="ta12")  # trans@a1 | trans@a2

    # ---------------- preamble: ta12, rhs2, e1 -> invC ----------------
    with tc.tile_pool(name="ps_pre", bufs=3, space="PSUM") as ps_pre:
        for fc in range(2):
            pst = ps_pre.tile([128, 512], f32, tag="pre")
            nc.tensor.matmul(
                pst[:, 0:2],
                transT_sb[:, fc * 128 : (fc + 1) * 128],
                a12_sb,
                start=True,
                stop=True,
            )
            nc.vector.tensor_copy(ta12[:, fc, :], pst[:, 0:2])
        nc.vector.tensor_copy(rhs2[:, :, 0:D], trans_sb)
        for fc in range(2):
            nc.vector.tensor_copy(rhs2[:, fc, D : D + 1], ta12[:, fc, 1:2])

        # e1 for own rows (piece 0 after host permutation) -> invC
        for h2 in range(2):
            pse = ps_pre.tile([128, 512], f32, tag="pre")
            for fc in range(2):
                nc.tensor.matmul(
                    pse[0:1, :],
                    ta12[:, fc, 0:1],
                    xt_tiles[0][:, fc, h2 * 512 : (h2 + 1) * 512],
                    start=(fc == 0),
                    stop=(fc == 1),
                )
            nc.scalar.activation(
                invc_row[0:1, h2 * 512 : (h2 + 1) * 512],
                pse[0:1, :],
                Exp,
                scale=-(1.0 - ALPHA),
            )
        # broadcast invC over all 128 partitions via K=1 matmul
        for h2 in range(2):
            psb = ps_pre.tile([128, 512], f32, tag="pre")
            nc.tensor.matmul(
                psb,
                ones128,
                invc_row[0:1, h2 * 512 : (h2 + 1) * 512],
                start=True,
                stop=True,
            )
            nc.vector.tensor_copy(invc_rep[:, h2 * 512 : (h2 + 1) * 512], psb)

    heps = ctx.enter_context(tc.tile_pool(name="heps", bufs=4, space="PSUM"))
    ps_acc = ctx.enter_context(tc.tile_pool(name="ps_acc", bufs=1, space="PSUM"))
    po0 = ps_acc.tile([D + 1, 512], f32, tag="po0")
    po1 = ps_acc.tile([D + 1, 512], f32, tag="po1")

    # ---------------- fused phase-1/phase-2 pipeline ----------------
    def phase1_step(k):
        piece = xt_tiles[k // 8]
        sub = k % 8
        ps_he = heps.tile([128, D + 1], f32, tag="he", bufs=4)
        for fc in range(2):
            nc.tensor.matmul(
                ps_he,
                piece[:, fc, sub * 128 : (sub + 1) * 128],
                rhs2[:, fc, :],
                start=(fc == 0),
                stop=(fc == 1),
            )
        nc.scalar.copy(haug[:, k, :], ps_he)  # f32 -> f16, e2 still in col D

    def piece_exps(p):
        sl = slice(p * 8, p * 8 + 8)
        nc.scalar.activation(d_all[:, sl], haug[:, sl, D], Exp, scale=1.0 - ALPHA)
        nc.scalar.activation(b2_all[:, sl], haug[:, sl, D], Exp, scale=ALPHA)
        for j in range(p * 8, p * 8 + 8):
            nc.vector.tensor_scalar(
                haug[:, j, 0:D], haug[:, j, 0:D], b2_all[:, j : j + 1], None, MULT
            )
        nc.vector.tensor_copy(haug[:, sl, D], b2_all[:, sl])

    def phase2_step(kk):
        m = mask_tiles[kk]
        v = vp.tile([128, R], SDT, tag="v", bufs=3)
        nc.vector.tensor_scalar(v, invc_rep, d_all[:, kk : kk + 1], None, MAX)
        eng = nc.gpsimd if (kk % GP_EVERY == GP_EVERY - 1) else nc.vector
        eng.tensor_tensor(v, v, m, MULT)
        nc.tensor.matmul(po0, haug[:, kk, :], v[:, 0:512], start=(kk == 0), stop=(kk == NCH - 1))
        nc.tensor.matmul(po1, haug[:, kk, :], v[:, 512:], start=(kk == 0), stop=(kk == NCH - 1))

    for k in range(NCH + LAG):
        if k < NCH:
            phase1_step(k)
            if k % 8 == 7:
                piece_exps(k // 8)
        if k >= LAG:
            phase2_step(k - LAG)

    # ---------------- normalize + store ----------------
    outp = ctx.enter_context(tc.tile_pool(name="outp", bufs=1))
    for h2, po in enumerate((po0, po1)):
        recip = outp.tile([1, 512], f32, tag="rc", bufs=2)
        nc.vector.reciprocal(recip, po[D : D + 1, :])
        rr = ps_acc.tile([D, 512], f32, tag="rr", bufs=2)
        nc.tensor.matmul(rr, ones64, recip, start=True, stop=True)
        rr_sb = outp.tile([D, 512], f32, tag="rrsb", bufs=2)
        nc.scalar.copy(rr_sb, rr)
        o_t = outp.tile([D, 512], f32, tag="ot", bufs=2)
        nc.vector.tensor_tensor(o_t, po[0:D, :], rr_sb, MULT)
        nc.gpsimd.dma_start(out=outT[:, h2 * 512 : (h2 + 1) * 512], in_=o_t)


def build_nc():
    nc = bacc.Bacc("TRN2", num_devices=N_CORES)
    xt = nc.dram_tensor("xt", [F_IN, N], f16, kind="ExternalInput")
    transp = nc.dram_tensor("transp", [F_IN, D], f16, kind="ExternalInput")
    transt = nc.dram_tensor("transt", [D, F_IN], f16, kind="ExternalInput")
    a12 = nc.dram_tensor("a12", [D, 2], f16, kind="ExternalInput")
    maskt = nc.dram_tensor("maskt", [N, R], MDT, kind="ExternalInput")
    outT = nc.dram_tensor("outT", [D, R], f32, kind="ExternalOutput")
    with ExitStack() as ctx:
        tc = ctx.enter_context(tile.TileContext(nc))
        build_kernel(
            ctx, tc, xt[:, :], transp[:, :], transt[:, :], a12[:, :], maskt[:, :], outT[:, :]
        )
    nc.compile()
    return nc


LAST_RESULTS = None


def kernel(x, mask, trans, attn, _trace=False):
    x = np.asarray(x, dtype=np.float32)
    mask = np.asarray(mask)
    trans = np.asarray(trans, dtype=np.float32)
    attn = np.asarray(attn, dtype=np.float32)

    xt16 = np.ascontiguousarray(x.T.astype(np.float16))
    transp16 = np.ascontiguousarray(trans.astype(np.float16))
    transt16 = np.ascontiguousarray(trans.T.astype(np.float16))
    a12h = np.ascontiguousarray(
        np.concatenate([attn[:D], attn[D:]], axis=1).astype(np.float16)
    )

    nc = build_nc()
    in_maps = []
    for c in range(N_CORES):
        perm = np.r_[c * R : (c + 1) * R, 0 : c * R, (c + 1) * R : N]
        mT = np.ascontiguousarray(
            mask[c * R : (c + 1) * R, :].T[perm].astype(MDT_NP)
        )
        in_maps.append(
            {
                "xt": np.ascontiguousarray(xt16[:, perm]),
                "transp": transp16,
                "transt": transt16,
                "a12": a12h,
                "maskt": mT,
            }
        )
    res = run_bass_kernel_spmd(nc, in_maps, list(range(N_CORES)), trace=_trace)
    global LAST_RESULTS
    LAST_RESULTS = res
    out = np.concatenate([res.results[c]["outT"].T for c in range(N_CORES)], axis=0)
    return np.ascontiguousarray(out, dtype=np.float32)


if __name__ == "__main__":
    nc = build_nc()
    print("built OK")
